# revision 1
# baseline (speedup 1.0000x reference)
"""GAT layer (single head) on 8 Trainium2 NeuronCores.

Strategy: destination-sharded edge parallelism.
  - Nodes padded to NPAD = 8*NB*128; core k owns NB blocks of 128 nodes.
  - Host sorts edges by (dst-core, src-chunk, dst-block) and pads each
    (block, chunk) run to whole tiles of 128 edges (capacity C_bc tiles,
    the max over all runs).
  - Device, per core:
      phase 1: zaug[n] = [z(n) | el(n) | er(n) | 1] for ALL nodes
               (z = h@W.T, el = z@a_l, er = z@a_r) via one fused matmul
               with WAUG = [W.T | wl | wr]; rows stored to DRAM tables
               (256B rows for the gather engine).
      phase 2: per edge tile of 128: dma_gather zaug[src] and er[dst]
               (core-local table, int16-safe); ex = exp(lrelu(el+er));
               one-hot-times-ex matrix via a single two-op tensor_scalar;
               Y[b] += [z|..|1].T @ ohx accumulated in PSUM per node
               block (numerator cols 0:32, denominator col 34);
               out = num / max(denom, eps).
    Softmax max-subtraction is dropped: |e| stays small for this model,
    so exp() is well-conditioned and the softmax ratio is unchanged.

  DRAM node tables use a tiled layout: node n lives at row
  (n % 128) * (NPAD/128) + n // 128, so phase 1 writes them with large
  contiguous per-partition DMA runs; the host bakes this mapping (and
  the 4-way int16 chunking of the z table) into the gather indices.
"""

import sys

sys.path.insert(0, "/opt/trn_rl_repo")

import numpy as np

import concourse.bacc as bacc
import concourse.bass as bass
import concourse.tile as tile
from concourse import mybir
from concourse.bass_utils import run_bass_kernel_spmd
from concourse.masks import make_identity

F32 = mybir.dt.float32
I16 = mybir.dt.int16

N_NODES = 100000
IN_FEATS = 128
OUT_FEATS = 32
NEG_SLOPE = 0.2
N_CORES = 8
BLK = 128
NB = 98  # blocks per core (full problem)
EL = 64  # table row: 64 f32 = 256B (dma_gather granularity)
NQ = 4  # int16 chunks of the z table
ZG = 512  # nodes per z-phase group
BGS = 14  # blocks per phase-2 group

C_EL = 32  # el column in zaug row
C_ER = 33  # er column
C_ONE = 34  # constant-one column

_cache = {}


def _build(C, nb=NB, bgs=BGS, dbg=False):
    """C = tiles of 128 edges per (block, chunk) run."""
    assert nb % bgs == 0, (nb, bgs)
    core_nodes = nb * BLK
    npad = N_CORES * core_nodes
    ncols = npad // BLK
    chunk_rows = (BLK // NQ) * ncols  # z-table rows per int16 chunk
    assert chunk_rows < 32768 and core_nodes < 32768
    nzg = npad // ZG
    sub = ZG // BLK
    T = NQ * nb * C  # tile columns per core
    NW = T * BLK // 16  # wrapped-index columns

    nc = bacc.Bacc("TRN2", target_bir_lowering=False, debug=False,
                   num_devices=N_CORES)

    hT = nc.dram_tensor("hT", [IN_FEATS, npad], F32, kind="ExternalInput")
    Wt = nc.dram_tensor("Wt", [OUT_FEATS, IN_FEATS], F32, kind="ExternalInput")
    av = nc.dram_tensor("av", [2 * OUT_FEATS, 1], F32, kind="ExternalInput")
    srcw = nc.dram_tensor("srcw", [BLK, NW], I16, kind="ExternalInput")
    erw = nc.dram_tensor("erw", [BLK, NW], I16, kind="ExternalInput")
    dstloc = nc.dram_tensor("dstloc", [BLK, T], F32, kind="ExternalInput")
    out = nc.dram_tensor("out", [core_nodes, OUT_FEATS], F32,
                         kind="ExternalOutput")

    zaug = nc.dram_tensor("zaug", [npad, EL], F32)
    ertab = nc.dram_tensor("ertab", [core_nodes, EL], F32)

    if dbg:
        NCOL0 = bgs * C
        zg_d = nc.dram_tensor("zg_d", [NQ, BLK, NCOL0, EL], F32,
                              kind="ExternalOutput")
        erg_d = nc.dram_tensor("erg_d", [NQ, BLK, NCOL0, EL], F32,
                               kind="ExternalOutput")
        ex_d = nc.dram_tensor("ex_d", [NQ, BLK, NCOL0], F32,
                              kind="ExternalOutput")
        y_d = nc.dram_tensor("y_d", [bgs, 64, BLK], F32,
                             kind="ExternalOutput")
        oh_d = nc.dram_tensor("oh_d", [BLK, BLK], F32, kind="ExternalOutput")
        ertab_d = nc.dram_tensor("ertab_d", [core_nodes, 1], F32,
                                 kind="ExternalOutput")

    with tile.TileContext(nc) as tc:
        with tc.tile_pool(name="const", bufs=1) as cpool:
            ident = cpool.tile([128, 128], F32)
            make_identity(nc, ident[:])
            iota = cpool.tile([128, BLK], F32)
            nc.gpsimd.iota(iota[:], pattern=[[1, BLK]], base=0,
                           channel_multiplier=0,
                           allow_small_or_imprecise_dtypes=True)

            # WAUG = [W.T | wl | wr]  (wl = W.T a_l, wr = W.T a_r)
            waug = cpool.tile([IN_FEATS, C_ONE], F32)
            nc.vector.memset(waug[:], 0.0)
            with tc.tile_pool(name="wprep", bufs=1) as wpool, \
                 tc.tile_pool(name="wpsum", bufs=2, space="PSUM") as wps:
                w_sb = wpool.tile([OUT_FEATS, IN_FEATS], F32)
                nc.sync.dma_start(out=w_sb[:], in_=Wt[:])
                al_sb = wpool.tile([OUT_FEATS, 1], F32)
                nc.sync.dma_start(out=al_sb[:], in_=av[0:OUT_FEATS, :])
                ar_sb = wpool.tile([OUT_FEATS, 1], F32)
                nc.sync.dma_start(out=ar_sb[:],
                                  in_=av[OUT_FEATS:2 * OUT_FEATS, :])
                wt_ps = wps.tile([IN_FEATS, OUT_FEATS], F32)
                nc.tensor.transpose(out=wt_ps[:], in_=w_sb[:],
                                    identity=ident[0:OUT_FEATS, 0:OUT_FEATS])
                nc.vector.tensor_copy(out=waug[:, 0:OUT_FEATS], in_=wt_ps[:])
                wl_ps = wps.tile([IN_FEATS, 1], F32)
                nc.tensor.matmul(out=wl_ps[:], lhsT=w_sb[:],
                                 rhs=al_sb[:], start=True, stop=True)
                nc.vector.tensor_copy(out=waug[:, C_EL:C_EL + 1],
                                      in_=wl_ps[:])
                wr_ps = wps.tile([IN_FEATS, 1], F32)
                nc.tensor.matmul(out=wr_ps[:], lhsT=w_sb[:],
                                 rhs=ar_sb[:], start=True, stop=True)
                nc.vector.tensor_copy(out=waug[:, C_ER:C_ER + 1],
                                      in_=wr_ps[:])

            # ---------------- phase 1: build zaug / ertab ----------------
            er_sb = cpool.tile([BLK, ncols], F32)
            zaug_t = zaug.ap().rearrange("(p c) z -> p c z", p=BLK)
            with tc.tile_pool(name="zh", bufs=3) as hpool, \
                 tc.tile_pool(name="zps", bufs=4, space="PSUM") as zps, \
                 tc.tile_pool(name="zrow", bufs=3) as rpool:
                for g in range(nzg):
                    n0 = g * ZG
                    htile = hpool.tile([IN_FEATS, ZG], F32)
                    nc.sync.dma_start(out=htile[:], in_=hT[:, n0:n0 + ZG])
                    zrows = rpool.tile([128, sub, EL], F32)
                    nc.vector.memset(zrows[:, :, C_ONE:C_ONE + 1], 1.0)
                    for s in range(sub):
                        z_ps = zps.tile([128, C_ONE], F32)
                        nc.tensor.matmul(
                            out=z_ps[:],
                            lhsT=htile[:, s * BLK:(s + 1) * BLK],
                            rhs=waug[:], start=True, stop=True)
                        nc.scalar.copy(out=zrows[:, s, 0:C_ONE], in_=z_ps[:])
                        nc.vector.tensor_copy(
                            out=er_sb[:, g * sub + s:g * sub + s + 1],
                            in_=z_ps[:, C_ER:C_ER + 1])
                    nc.sync.dma_start(
                        out=zaug_t[:, g * sub:(g + 1) * sub, :],
                        in_=zrows[:])
                pid = nc.gpsimd.partition_id()
                ertab_t = ertab.ap().rearrange("(p b) e -> p b e", p=BLK)
                nc.gpsimd.dma_start(
                    out=ertab_t[:, :, 0:1],
                    in_=er_sb[:, bass.ts(pid, nb), None])
                if dbg:
                    nc.gpsimd.dma_start(
                        out=ertab_d.ap().rearrange("(p b) e -> p b e",
                                                   p=BLK),
                        in_=er_sb[:, bass.ts(pid, nb), None])

            # ---------------- phase 2: edges ----------------
            with tc.tile_pool(name="ix", bufs=1) as ixpool:
                srcw_sb = ixpool.tile([BLK, NW], I16)
                nc.sync.dma_start(out=srcw_sb[:], in_=srcw[:])
                erw_sb = ixpool.tile([BLK, NW], I16)
                nc.sync.dma_start(out=erw_sb[:], in_=erw[:])
                dl_sb = ixpool.tile([BLK, T], F32)
                nc.sync.dma_start(out=dl_sb[:], in_=dstloc[:])

                with tc.tile_pool(name="zg", bufs=3) as zgpool, \
                     tc.tile_pool(name="erg", bufs=3) as erpool, \
                     tc.tile_pool(name="ex", bufs=4) as expool, \
                     tc.tile_pool(name="oh", bufs=12) as ohpool, \
                     tc.tile_pool(name="acc", bufs=2) as apool, \
                     tc.tile_pool(name="yps", bufs=5, space="PSUM") as ypool, \
                     tc.tile_pool(name="ytp", bufs=3, space="PSUM") as ytpool, \
                     tc.tile_pool(name="fin", bufs=6) as fpool, \
                     tc.tile_pool(name="ost", bufs=2) as opool:
                    NCOL = bgs * C  # tile columns per (group, chunk)
                    NY = C_ONE + 1
                    for bg in range(nb // bgs):
                        acc = apool.tile([NY, bgs, BLK], F32)
                        nc.vector.memset(acc[:], 0.0)
                        for q in range(NQ):
                            colbase = q * nb * C + bg * NCOL
                            nidx = NCOL * BLK
                            w0 = colbase * BLK // 16
                            zg = zgpool.tile([BLK, NCOL, EL], F32)
                            erg = erpool.tile([BLK, NCOL, EL], F32)
                            # SWDGE ring cap: <=1024 indices per call
                            GCH = 8  # tile-columns per call (1024 idxs)
                            for j0 in range(0, NCOL, GCH):
                                j1 = min(j0 + GCH, NCOL)
                                ni = (j1 - j0) * BLK
                                wj = w0 + j0 * BLK // 16
                                nc.gpsimd.dma_gather(
                                    out_ap=zg[:, j0:j1, :],
                                    in_ap=zaug[q * chunk_rows:
                                               (q + 1) * chunk_rows, :],
                                    idxs_ap=srcw_sb[:, wj:wj + ni // 16],
                                    num_idxs=ni, num_idxs_reg=ni,
                                    elem_size=EL)
                                nc.gpsimd.dma_gather(
                                    out_ap=erg[:, j0:j1, :], in_ap=ertab[:],
                                    idxs_ap=erw_sb[:, wj:wj + ni // 16],
                                    num_idxs=ni, num_idxs_reg=ni,
                                    elem_size=EL)
                            ex = expool.tile([BLK, NCOL], F32)
                            sv = expool.tile([BLK, NCOL], F32, tag="sv")
                            nc.vector.tensor_add(out=sv[:],
                                                 in0=zg[:, :, C_EL],
                                                 in1=erg[:, :, 0])
                            # leaky_relu(x) = max(x, 0.2x); the Lrelu ACT
                            # table has a baked-in 0.01 slope, so do it
                            # manually
                            nc.vector.tensor_scalar(
                                ex[:], sv[:], NEG_SLOPE, None,
                                mybir.AluOpType.mult)
                            nc.vector.tensor_tensor(
                                out=ex[:], in0=ex[:], in1=sv[:],
                                op=mybir.AluOpType.max)
                            nc.scalar.activation(
                                out=ex[:], in_=ex[:],
                                func=mybir.ActivationFunctionType.Exp)
                            if dbg and bg == 0:
                                nc.sync.dma_start(out=zg_d[q], in_=zg[:])
                                nc.sync.dma_start(out=erg_d[q], in_=erg[:])
                                nc.sync.dma_start(out=ex_d[q], in_=ex[:])
                            for b in range(bgs):
                                y_ps = ypool.tile([NY, BLK], F32)
                                for t in range(C):
                                    lcol = b * C + t
                                    col = colbase + lcol
                                    oh = ohpool.tile([BLK, BLK], F32)
                                    nc.vector.tensor_scalar(
                                        oh[:], iota[:],
                                        dl_sb[:, col:col + 1],
                                        ex[:, lcol:lcol + 1],
                                        mybir.AluOpType.is_equal,
                                        mybir.AluOpType.mult)
                                    if dbg and bg == 0 and q == 0 and b == 0 \
                                            and t == 0:
                                        nc.sync.dma_start(out=oh_d[:],
                                                          in_=oh[:])
                                    nc.tensor.matmul(
                                        out=y_ps[:],
                                        lhsT=zg[:, lcol, 0:NY],
                                        rhs=oh[:],
                                        start=(t == 0),
                                        stop=(t == C - 1))
                                nc.vector.tensor_add(out=acc[:, b, :],
                                                     in0=acc[:, b, :],
                                                     in1=y_ps[:])
                        ost = opool.tile([BLK, bgs, OUT_FEATS], F32)
                        for b in range(bgs):
                            if dbg and bg == 0:
                                nc.sync.dma_start(out=y_d[b, 0:NY, :],
                                                  in_=acc[:, b, :])
                            yt = ytpool.tile([BLK, NY], F32)
                            nc.tensor.transpose(out=yt[:], in_=acc[:, b, :],
                                                identity=ident[0:NY, 0:NY])
                            den = fpool.tile([BLK, 1], F32)
                            nc.vector.tensor_scalar(
                                den[:], yt[:, C_ONE:C_ONE + 1], 1e-16, None,
                                mybir.AluOpType.max)
                            rden = fpool.tile([BLK, 1], F32)
                            nc.vector.reciprocal(out=rden[:], in_=den[:])
                            nc.vector.tensor_scalar(
                                ost[:, b, :], yt[:, 0:OUT_FEATS], rden[:],
                                None, mybir.AluOpType.mult)
                        n0 = bg * bgs * BLK
                        nc.sync.dma_start(
                            out=out[n0:n0 + bgs * BLK, :].rearrange(
                                "(s p) c -> p s c", p=BLK),
                            in_=ost[:])

    nc.compile()
    return nc


def _prep(h, W, a, src, dst, nb=NB, n_nodes=N_NODES):
    """Host-side sharding / index layout (integer index manipulation and
    zero-padding only - all floating-point math runs on device)."""
    core_nodes = nb * BLK
    npad = N_CORES * core_nodes
    ncols = npad // BLK
    chunk_rows = (BLK // NQ) * ncols

    h = np.asarray(h, dtype=np.float32)
    W = np.ascontiguousarray(np.asarray(W, dtype=np.float32))
    a = np.asarray(a, dtype=np.float32).reshape(-1)
    src = np.asarray(src, dtype=np.int64)
    dst = np.asarray(dst, dtype=np.int64)

    hT = np.zeros((IN_FEATS, npad), dtype=np.float32)
    hT[:, :n_nodes] = h.T
    av = np.ascontiguousarray(a.reshape(-1, 1), dtype=np.float32)

    core = dst // core_nodes
    b_of = (dst % core_nodes) // BLK
    q_of = (src % BLK) // (BLK // NQ)
    grp = (core * NQ + q_of) * nb + b_of
    order = np.argsort(grp, kind="stable")
    gs = grp[order]
    ss = src[order]
    ds = dst[order]

    counts = np.bincount(gs, minlength=N_CORES * NQ * nb)
    C = int(max(1, -(-counts.max() // BLK)))
    T = NQ * nb * C
    NW = T * BLK // 16

    # global slot of each sorted edge
    starts = np.zeros(N_CORES * NQ * nb + 1, dtype=np.int64)
    np.cumsum(counts, out=starts[1:])
    rank = np.arange(len(ss)) - starts[gs]
    # within-core group index: (q * nb + b) for that core
    gloc = gs % (NQ * nb)
    slot = gloc * (C * BLK) + rank  # slot within the core's edge buffer

    src_t = (ss % BLK) * ncols + ss // BLK  # tiled z-table row
    src_i16 = (src_t - q_of[order] * chunk_rows).astype(np.int16)
    er_i16_all = ((ds % BLK) * nb + (ds % core_nodes) // BLK).astype(np.int16)
    dl_all = (ds % core_nodes - b_of[order] * BLK).astype(np.float32)

    srcw = np.zeros((N_CORES, BLK, NW), dtype=np.int16)
    erw = np.zeros((N_CORES, BLK, NW), dtype=np.int16)
    dstloc = np.full((N_CORES, BLK, T), -1.0, dtype=np.float32)
    for k in range(N_CORES):
        m = core[order] == k
        sl = slot[m]
        sflat = np.zeros(T * BLK, dtype=np.int16)
        eflat = np.zeros(T * BLK, dtype=np.int16)
        dflat = np.full(T * BLK, -1.0, dtype=np.float32)
        sflat[sl] = src_i16[m]
        eflat[sl] = er_i16_all[m]
        dflat[sl] = dl_all[m]
        # wrapped-16, replicated over the 8 gpsimd groups
        srcw[k] = np.tile(sflat.reshape(-1, 16).T, (8, 1))
        erw[k] = np.tile(eflat.reshape(-1, 16).T, (8, 1))
        dstloc[k] = dflat.reshape(T, BLK).T
    return hT, W, av, srcw, erw, dstloc, C


def kernel(h, W, a, src, dst):
    hT, Wm, av, srcw, erw, dstloc, C = _prep(h, W, a, src, dst)
    if C not in _cache:
        _cache[C] = _build(C)
    nc = _cache[C]
    in_maps = []
    for k in range(N_CORES):
        in_maps.append({
            "hT": hT,
            "Wt": Wm,
            "av": av,
            "srcw": srcw[k],
            "erw": erw[k],
            "dstloc": dstloc[k],
        })
    res = run_bass_kernel_spmd(nc, in_maps, list(range(N_CORES)))
    outs = [res.results[k]["out"] for k in range(N_CORES)]
    full = np.concatenate(outs, axis=0)[:N_NODES]
    return np.ascontiguousarray(full, dtype=np.float32)



# revision 5
# speedup vs baseline: 18.8930x; 18.8930x over previous
"""GAT layer (single head) on 8 Trainium2 NeuronCores.

Strategy: destination-sharded edge parallelism.
  - Nodes padded to NPAD = 8*NB*128; core k owns NB blocks of 128 nodes.
  - Host sorts edges by (dst-core, src-chunk, dst-block) and pads each
    (block, chunk) run to whole tiles of 128 edges (capacity C_bc tiles,
    the max over all runs).
  - Device, per core:
      phase 1: zaug[n] = [z(n) | el(n) | er(n) | 1] for ALL nodes
               (z = h@W.T, el = z@a_l, er = z@a_r) via one fused matmul
               with WAUG = [W.T | wl | wr]; rows stored to DRAM tables
               (256B rows for the gather engine).
      phase 2: per edge tile of 128: dma_gather zaug[src] and er[dst]
               (core-local table, int16-safe); ex = exp(lrelu(el+er));
               one-hot-times-ex matrix via a single two-op tensor_scalar;
               Y[b] += [z|..|1].T @ ohx accumulated in PSUM per node
               block (numerator cols 0:32, denominator col 34);
               out = num / max(denom, eps).
    Softmax max-subtraction is dropped: |e| stays small for this model,
    so exp() is well-conditioned and the softmax ratio is unchanged.

  DRAM node tables use a tiled layout: node n lives at row
  (n % 128) * (NPAD/128) + n // 128, so phase 1 writes them with large
  contiguous per-partition DMA runs; the host bakes this mapping (and
  the 4-way int16 chunking of the z table) into the gather indices.
"""

import sys

sys.path.insert(0, "/opt/trn_rl_repo")

import numpy as np

import concourse.bacc as bacc
import concourse.bass as bass
import concourse.tile as tile
from concourse import mybir
from concourse.bass_utils import run_bass_kernel_spmd
from concourse.masks import make_identity

F32 = mybir.dt.float32
I16 = mybir.dt.int16

N_NODES = 100000
IN_FEATS = 128
OUT_FEATS = 32
NEG_SLOPE = 0.2
N_CORES = 8
BLK = 128
NB = 98  # blocks per core (full problem)
EL = 64  # table row: 64 f32 = 256B (dma_gather granularity)
NQ = 4  # int16 chunks of the z table
ZG = 512  # nodes per z-phase group
BGS = 14  # blocks per phase-2 group

C_EL = 32  # el column in zaug row
C_ER = 33  # er column
C_ONE = 34  # constant-one column

_cache = {}


def _build(C, nb=NB, bgs=BGS, dbg=False, reps=1):
    """C = tiles of 128 edges per (block, chunk) run.

    reps > 1 wraps the whole body in a hardware For_i loop that re-executes
    the kernel reps times back-to-back — used by the timing harness to
    amortize the host-dispatch floor out of the per-execution measurement."""
    assert nb % bgs == 0, (nb, bgs)
    core_nodes = nb * BLK
    npad = N_CORES * core_nodes
    ncols = npad // BLK
    chunk_rows = (BLK // NQ) * ncols  # z-table rows per int16 chunk
    assert chunk_rows < 32768 and core_nodes < 32768
    nzg = npad // ZG
    sub = ZG // BLK
    T = NQ * nb * C  # tile columns per core
    NW = T * BLK // 16  # wrapped-index columns

    nc = bacc.Bacc("TRN2", target_bir_lowering=False, debug=False,
                   num_devices=N_CORES)

    hT = nc.dram_tensor("hT", [IN_FEATS, npad], F32, kind="ExternalInput")
    Wt = nc.dram_tensor("Wt", [OUT_FEATS, IN_FEATS], F32, kind="ExternalInput")
    av = nc.dram_tensor("av", [2 * OUT_FEATS, 1], F32, kind="ExternalInput")
    srcw = nc.dram_tensor("srcw", [BLK, NW], I16, kind="ExternalInput")
    erw = nc.dram_tensor("erw", [BLK, NW], I16, kind="ExternalInput")
    dstloc = nc.dram_tensor("dstloc", [BLK, T], F32, kind="ExternalInput")
    out = nc.dram_tensor("out", [core_nodes, OUT_FEATS], F32,
                         kind="ExternalOutput")

    zaug = nc.dram_tensor("zaug", [npad, EL], F32)
    ertab = nc.dram_tensor("ertab", [core_nodes, EL], F32)

    if dbg:
        NCOL0 = bgs * C
        zg_d = nc.dram_tensor("zg_d", [NQ, BLK, NCOL0, EL], F32,
                              kind="ExternalOutput")
        erg_d = nc.dram_tensor("erg_d", [NQ, BLK, NCOL0, EL], F32,
                               kind="ExternalOutput")
        ex_d = nc.dram_tensor("ex_d", [NQ, BLK, NCOL0], F32,
                              kind="ExternalOutput")
        y_d = nc.dram_tensor("y_d", [bgs, 64, BLK], F32,
                             kind="ExternalOutput")
        oh_d = nc.dram_tensor("oh_d", [BLK, BLK], F32, kind="ExternalOutput")
        ertab_d = nc.dram_tensor("ertab_d", [core_nodes, 1], F32,
                                 kind="ExternalOutput")

    import contextlib

    with tile.TileContext(nc) as tc:
        rep_ctx = tc.For_i(0, reps) if reps > 1 else contextlib.nullcontext()
        with rep_ctx, tc.tile_pool(name="const", bufs=1) as cpool:
            ident = cpool.tile([128, 128], F32)
            make_identity(nc, ident[:])
            iota = cpool.tile([128, BLK], F32)
            nc.gpsimd.iota(iota[:], pattern=[[1, BLK]], base=0,
                           channel_multiplier=0,
                           allow_small_or_imprecise_dtypes=True)

            # WAUG = [W.T | wl | wr]  (wl = W.T a_l, wr = W.T a_r)
            waug = cpool.tile([IN_FEATS, C_ONE], F32)
            nc.vector.memset(waug[:], 0.0)
            with tc.tile_pool(name="wprep", bufs=1) as wpool, \
                 tc.tile_pool(name="wpsum", bufs=2, space="PSUM") as wps:
                w_sb = wpool.tile([OUT_FEATS, IN_FEATS], F32)
                nc.sync.dma_start(out=w_sb[:], in_=Wt[:])
                al_sb = wpool.tile([OUT_FEATS, 1], F32)
                nc.sync.dma_start(out=al_sb[:], in_=av[0:OUT_FEATS, :])
                ar_sb = wpool.tile([OUT_FEATS, 1], F32)
                nc.sync.dma_start(out=ar_sb[:],
                                  in_=av[OUT_FEATS:2 * OUT_FEATS, :])
                wt_ps = wps.tile([IN_FEATS, OUT_FEATS], F32)
                nc.tensor.transpose(out=wt_ps[:], in_=w_sb[:],
                                    identity=ident[0:OUT_FEATS, 0:OUT_FEATS])
                nc.vector.tensor_copy(out=waug[:, 0:OUT_FEATS], in_=wt_ps[:])
                wl_ps = wps.tile([IN_FEATS, 1], F32)
                nc.tensor.matmul(out=wl_ps[:], lhsT=w_sb[:],
                                 rhs=al_sb[:], start=True, stop=True)
                nc.vector.tensor_copy(out=waug[:, C_EL:C_EL + 1],
                                      in_=wl_ps[:])
                wr_ps = wps.tile([IN_FEATS, 1], F32)
                nc.tensor.matmul(out=wr_ps[:], lhsT=w_sb[:],
                                 rhs=ar_sb[:], start=True, stop=True)
                nc.vector.tensor_copy(out=waug[:, C_ER:C_ER + 1],
                                      in_=wr_ps[:])

            # ---------------- phase 1: build zaug / ertab ----------------
            er_sb = cpool.tile([BLK, ncols], F32)
            zaug_t = zaug.ap().rearrange("(p c) z -> p c z", p=BLK)
            with tc.tile_pool(name="zh", bufs=3) as hpool, \
                 tc.tile_pool(name="zps", bufs=4, space="PSUM") as zps, \
                 tc.tile_pool(name="zrow", bufs=3) as rpool:
                for g in range(nzg):
                    n0 = g * ZG
                    htile = hpool.tile([IN_FEATS, ZG], F32)
                    nc.sync.dma_start(out=htile[:], in_=hT[:, n0:n0 + ZG])
                    zrows = rpool.tile([128, sub, EL], F32)
                    nc.vector.memset(zrows[:, :, C_ONE:C_ONE + 1], 1.0)
                    for s in range(sub):
                        z_ps = zps.tile([128, C_ONE], F32)
                        nc.tensor.matmul(
                            out=z_ps[:],
                            lhsT=htile[:, s * BLK:(s + 1) * BLK],
                            rhs=waug[:], start=True, stop=True)
                        nc.scalar.copy(out=zrows[:, s, 0:C_ONE], in_=z_ps[:])
                        nc.vector.tensor_copy(
                            out=er_sb[:, g * sub + s:g * sub + s + 1],
                            in_=z_ps[:, C_ER:C_ER + 1])
                    nc.sync.dma_start(
                        out=zaug_t[:, g * sub:(g + 1) * sub, :],
                        in_=zrows[:])
                pid = nc.gpsimd.partition_id()
                ertab_t = ertab.ap().rearrange("(p b) e -> p b e", p=BLK)
                nc.gpsimd.dma_start(
                    out=ertab_t[:, :, 0:1],
                    in_=er_sb[:, bass.ts(pid, nb), None])
                if dbg:
                    nc.gpsimd.dma_start(
                        out=ertab_d.ap().rearrange("(p b) e -> p b e",
                                                   p=BLK),
                        in_=er_sb[:, bass.ts(pid, nb), None])

            # ---------------- phase 2: edges ----------------
            with tc.tile_pool(name="ix", bufs=1) as ixpool:
                srcw_sb = ixpool.tile([BLK, NW], I16)
                nc.sync.dma_start(out=srcw_sb[:], in_=srcw[:])
                erw_sb = ixpool.tile([BLK, NW], I16)
                nc.sync.dma_start(out=erw_sb[:], in_=erw[:])
                dl_sb = ixpool.tile([BLK, T], F32)
                nc.sync.dma_start(out=dl_sb[:], in_=dstloc[:])

                with tc.tile_pool(name="zg", bufs=3) as zgpool, \
                     tc.tile_pool(name="erg", bufs=3) as erpool, \
                     tc.tile_pool(name="ex", bufs=4) as expool, \
                     tc.tile_pool(name="oh", bufs=12) as ohpool, \
                     tc.tile_pool(name="acc", bufs=2) as apool, \
                     tc.tile_pool(name="yps", bufs=5, space="PSUM") as ypool, \
                     tc.tile_pool(name="ytp", bufs=3, space="PSUM") as ytpool, \
                     tc.tile_pool(name="fin", bufs=6) as fpool, \
                     tc.tile_pool(name="ost", bufs=2) as opool:
                    NCOL = bgs * C  # tile columns per (group, chunk)
                    NY = C_ONE + 1
                    for bg in range(nb // bgs):
                        acc = apool.tile([NY, bgs, BLK], F32)
                        nc.vector.memset(acc[:], 0.0)
                        for q in range(NQ):
                            colbase = q * nb * C + bg * NCOL
                            nidx = NCOL * BLK
                            w0 = colbase * BLK // 16
                            zg = zgpool.tile([BLK, NCOL, EL], F32)
                            erg = erpool.tile([BLK, NCOL, EL], F32)
                            # SWDGE ring cap: <=1024 indices per call
                            GCH = 8  # tile-columns per call (1024 idxs)
                            for j0 in range(0, NCOL, GCH):
                                j1 = min(j0 + GCH, NCOL)
                                ni = (j1 - j0) * BLK
                                wj = w0 + j0 * BLK // 16
                                nc.gpsimd.dma_gather(
                                    out_ap=zg[:, j0:j1, :],
                                    in_ap=zaug[q * chunk_rows:
                                               (q + 1) * chunk_rows, :],
                                    idxs_ap=srcw_sb[:, wj:wj + ni // 16],
                                    num_idxs=ni, num_idxs_reg=ni,
                                    elem_size=EL)
                                nc.gpsimd.dma_gather(
                                    out_ap=erg[:, j0:j1, :], in_ap=ertab[:],
                                    idxs_ap=erw_sb[:, wj:wj + ni // 16],
                                    num_idxs=ni, num_idxs_reg=ni,
                                    elem_size=EL)
                            ex = expool.tile([BLK, NCOL], F32)
                            sv = expool.tile([BLK, NCOL], F32, tag="sv")
                            nc.vector.tensor_add(out=sv[:],
                                                 in0=zg[:, :, C_EL],
                                                 in1=erg[:, :, 0])
                            # leaky_relu(x) = max(x, 0.2x); the Lrelu ACT
                            # table has a baked-in 0.01 slope, so do it
                            # manually
                            nc.vector.tensor_scalar(
                                ex[:], sv[:], NEG_SLOPE, None,
                                mybir.AluOpType.mult)
                            nc.vector.tensor_tensor(
                                out=ex[:], in0=ex[:], in1=sv[:],
                                op=mybir.AluOpType.max)
                            nc.scalar.activation(
                                out=ex[:], in_=ex[:],
                                func=mybir.ActivationFunctionType.Exp)
                            if dbg and bg == 0:
                                nc.sync.dma_start(out=zg_d[q], in_=zg[:])
                                nc.sync.dma_start(out=erg_d[q], in_=erg[:])
                                nc.sync.dma_start(out=ex_d[q], in_=ex[:])
                            for b in range(bgs):
                                y_ps = ypool.tile([NY, BLK], F32)
                                for t in range(C):
                                    lcol = b * C + t
                                    col = colbase + lcol
                                    oh = ohpool.tile([BLK, BLK], F32)
                                    nc.vector.tensor_scalar(
                                        oh[:], iota[:],
                                        dl_sb[:, col:col + 1],
                                        ex[:, lcol:lcol + 1],
                                        mybir.AluOpType.is_equal,
                                        mybir.AluOpType.mult)
                                    if dbg and bg == 0 and q == 0 and b == 0 \
                                            and t == 0:
                                        nc.sync.dma_start(out=oh_d[:],
                                                          in_=oh[:])
                                    nc.tensor.matmul(
                                        out=y_ps[:],
                                        lhsT=zg[:, lcol, 0:NY],
                                        rhs=oh[:],
                                        start=(t == 0),
                                        stop=(t == C - 1))
                                nc.vector.tensor_add(out=acc[:, b, :],
                                                     in0=acc[:, b, :],
                                                     in1=y_ps[:])
                        ost = opool.tile([BLK, bgs, OUT_FEATS], F32)
                        for b in range(bgs):
                            if dbg and bg == 0:
                                nc.sync.dma_start(out=y_d[b, 0:NY, :],
                                                  in_=acc[:, b, :])
                            yt = ytpool.tile([BLK, NY], F32)
                            nc.tensor.transpose(out=yt[:], in_=acc[:, b, :],
                                                identity=ident[0:NY, 0:NY])
                            den = fpool.tile([BLK, 1], F32)
                            nc.vector.tensor_scalar(
                                den[:], yt[:, C_ONE:C_ONE + 1], 1e-16, None,
                                mybir.AluOpType.max)
                            rden = fpool.tile([BLK, 1], F32)
                            nc.vector.reciprocal(out=rden[:], in_=den[:])
                            nc.vector.tensor_scalar(
                                ost[:, b, :], yt[:, 0:OUT_FEATS], rden[:],
                                None, mybir.AluOpType.mult)
                        n0 = bg * bgs * BLK
                        nc.sync.dma_start(
                            out=out[n0:n0 + bgs * BLK, :].rearrange(
                                "(s p) c -> p s c", p=BLK),
                            in_=ost[:])

    nc.compile()
    return nc


def _prep(h, W, a, src, dst, nb=NB, n_nodes=N_NODES):
    """Host-side sharding / index layout (integer index manipulation and
    zero-padding only - all floating-point math runs on device)."""
    core_nodes = nb * BLK
    npad = N_CORES * core_nodes
    ncols = npad // BLK
    chunk_rows = (BLK // NQ) * ncols

    h = np.asarray(h, dtype=np.float32)
    W = np.ascontiguousarray(np.asarray(W, dtype=np.float32))
    a = np.asarray(a, dtype=np.float32).reshape(-1)
    src = np.asarray(src, dtype=np.int64)
    dst = np.asarray(dst, dtype=np.int64)

    hT = np.zeros((IN_FEATS, npad), dtype=np.float32)
    hT[:, :n_nodes] = h.T
    av = np.ascontiguousarray(a.reshape(-1, 1), dtype=np.float32)

    core = dst // core_nodes
    b_of = (dst % core_nodes) // BLK
    q_of = (src % BLK) // (BLK // NQ)
    grp = (core * NQ + q_of) * nb + b_of
    order = np.argsort(grp, kind="stable")
    gs = grp[order]
    ss = src[order]
    ds = dst[order]

    counts = np.bincount(gs, minlength=N_CORES * NQ * nb)
    C = int(max(1, -(-counts.max() // BLK)))
    T = NQ * nb * C
    NW = T * BLK // 16

    # global slot of each sorted edge
    starts = np.zeros(N_CORES * NQ * nb + 1, dtype=np.int64)
    np.cumsum(counts, out=starts[1:])
    rank = np.arange(len(ss)) - starts[gs]
    # within-core group index: (q * nb + b) for that core
    gloc = gs % (NQ * nb)
    slot = gloc * (C * BLK) + rank  # slot within the core's edge buffer

    src_t = (ss % BLK) * ncols + ss // BLK  # tiled z-table row
    src_i16 = (src_t - q_of[order] * chunk_rows).astype(np.int16)
    er_i16_all = ((ds % BLK) * nb + (ds % core_nodes) // BLK).astype(np.int16)
    dl_all = (ds % core_nodes - b_of[order] * BLK).astype(np.float32)

    srcw = np.zeros((N_CORES, BLK, NW), dtype=np.int16)
    erw = np.zeros((N_CORES, BLK, NW), dtype=np.int16)
    dstloc = np.full((N_CORES, BLK, T), -1.0, dtype=np.float32)
    for k in range(N_CORES):
        m = core[order] == k
        sl = slot[m]
        sflat = np.zeros(T * BLK, dtype=np.int16)
        eflat = np.zeros(T * BLK, dtype=np.int16)
        dflat = np.full(T * BLK, -1.0, dtype=np.float32)
        sflat[sl] = src_i16[m]
        eflat[sl] = er_i16_all[m]
        dflat[sl] = dl_all[m]
        # wrapped-16, replicated over the 8 gpsimd groups
        srcw[k] = np.tile(sflat.reshape(-1, 16).T, (8, 1))
        erw[k] = np.tile(eflat.reshape(-1, 16).T, (8, 1))
        dstloc[k] = dflat.reshape(T, BLK).T
    return hT, W, av, srcw, erw, dstloc, C


def _in_maps(prep):
    hT, Wm, av, srcw, erw, dstloc = prep[:-1]
    return [
        {"hT": hT, "Wt": Wm, "av": av, "srcw": srcw[k], "erw": erw[k],
         "dstloc": dstloc[k]}
        for k in range(N_CORES)
    ]


def kernel(h, W, a, src, dst):
    prep = _prep(h, W, a, src, dst)
    C = prep[-1]
    if (C, 1) not in _cache:
        _cache[C, 1] = _build(C)
    nc = _cache[C, 1]
    in_maps = _in_maps(prep)
    res = run_bass_kernel_spmd(nc, in_maps, list(range(N_CORES)))
    outs = [res.results[k]["out"] for k in range(N_CORES)]
    full = np.concatenate(outs, axis=0)[:N_NODES]
    return np.ascontiguousarray(full, dtype=np.float32)



# revision 15
# speedup vs baseline: 38.6079x; 2.0435x over previous
"""GAT layer (single head) on 8 Trainium2 NeuronCores.

Strategy: destination-sharded edge parallelism.
  - Nodes padded to NPAD = 8*NB*128; core k owns NB blocks of 128 nodes.
  - Host sorts edges by (dst-core, src-chunk, dst-block) and pads each
    (block, chunk) run to whole tiles of 128 edges (capacity C tiles, the
    max over all runs); within each run edges are ordered by z-table row
    so each gather call hits DRAM in ascending-address order.
  - Device, per core:
      phase 1: for ALL nodes compute [z | el | er] = h @ [W.T | wl | wr]
               (wl = W.T a_l, wr = W.T a_r) and store rows to a DRAM
               table zaug: 256B rows of 128 bf16 lanes
               [z:32 bf16 | one bf16 | pad | el f32 (2 lanes) | pad...].
               er is kept on-chip only: er_sb [128, ncols] in SBUF.
      phase 2: per edge tile of 128: dma_gather zaug[src] (the ONLY
               per-edge DRAM traffic); er[dst] is reconstructed on-chip:
               a K=1 matmul broadcasts the tile's dst indices across
               partitions, is_equal against a per-partition iota gives the
               node-major one-hot ohpT, and ohpT.T @ er_col is er per
               edge. ex = exp(lrelu(el+er)); oh = onehot(dst)*ex (bf16);
               Y[b] += [z|1].T @ oh accumulated in PSUM per node block
               (numerator cols 0:32, denominator col 32);
               out = num / max(denom, eps).
    Softmax max-subtraction is dropped: |e| stays small for this model,
    so exp() is well-conditioned and the softmax ratio is unchanged.

  DRAM node table uses a tiled layout: node n lives at row
  (n % 128) * (NPAD/128) + n // 128, so phase 1 writes it with large
  contiguous per-partition DMA runs; the host bakes this mapping (and
  the 4-way int16 chunking) into the gather indices.

  reps > 1 wraps the whole body in a hardware For_i loop that re-executes
  the kernel reps times back-to-back — used by the timing harness to
  amortize the host-dispatch floor out of the per-execution measurement.
"""

import contextlib
import sys

sys.path.insert(0, "/opt/trn_rl_repo")

import numpy as np

import concourse.bacc as bacc
import concourse.bass as bass
import concourse.tile as tile
from concourse import mybir
from concourse.bass_utils import run_bass_kernel_spmd
from concourse.masks import make_identity

F32 = mybir.dt.float32
BF16 = mybir.dt.bfloat16
I16 = mybir.dt.int16

N_NODES = 100000
IN_FEATS = 128
OUT_FEATS = 32
NEG_SLOPE = 0.2
N_CORES = 8
BLK = 128
NB = 98  # blocks per core (full problem)
NQ = 4  # int16 chunks of the z table
ZG = 512  # nodes per z-phase group

EL = 128  # zaug row: 128 bf16 lanes = 256B (dma_gather granularity)
C_ONE = 32  # constant-one lane (bf16)
C_ELF = 34  # el as f32 at bf16 lanes 34:36
NY = 33  # scatter matmul lhsT width: z 0:32 + one @32

_cache = {}


def _build(C, nb=NB, reps=1):
    """C = tiles of 128 edges per (block, chunk) run."""
    core_nodes = nb * BLK
    npad = N_CORES * core_nodes
    ncols = npad // BLK
    chunk_rows = (BLK // NQ) * ncols  # z-table rows per int16 chunk
    assert chunk_rows < 32768 and core_nodes < 32768
    nzg = npad // ZG
    sub = ZG // BLK
    T = NQ * nb * C  # tile columns per core
    NW = T * BLK // 16  # wrapped-index columns

    nc = bacc.Bacc("TRN2", target_bir_lowering=False, debug=False,
                   num_devices=N_CORES)

    hT = nc.dram_tensor("hT", [IN_FEATS, npad], F32, kind="ExternalInput")
    Wt = nc.dram_tensor("Wt", [OUT_FEATS, IN_FEATS], F32, kind="ExternalInput")
    av = nc.dram_tensor("av", [2 * OUT_FEATS, 1], F32, kind="ExternalInput")
    srcw = nc.dram_tensor("srcw", [BLK, NW], I16, kind="ExternalInput")
    dstcol = nc.dram_tensor("dstcol", [BLK, T], F32, kind="ExternalInput")
    # dst-local index per edge slot, row-major [tile, slot]
    dstrow = nc.dram_tensor("dstrow", [T, BLK], BF16, kind="ExternalInput")
    # valid (non-padding) edges per (q, b) run, for SWDGE trailing-skip
    gcnt = nc.dram_tensor("gcnt", [1, NQ * nb], mybir.dt.int32,
                          kind="ExternalInput")
    out = nc.dram_tensor("out", [core_nodes, OUT_FEATS], F32,
                         kind="ExternalOutput")

    zaug = nc.dram_tensor("zaug", [npad, EL], BF16)

    with tile.TileContext(nc) as tc:
        rep_ctx = tc.For_i(0, reps) if reps > 1 else contextlib.nullcontext()
        with rep_ctx, tc.tile_pool(name="const", bufs=1) as cpool:
            ident = cpool.tile([128, 128], F32)
            make_identity(nc, ident[:])
            iota_bf = cpool.tile([128, BLK], BF16)
            nc.gpsimd.iota(iota_bf[:], pattern=[[1, BLK]], base=0,
                           channel_multiplier=0,
                           allow_small_or_imprecise_dtypes=True)
            iota_p = cpool.tile([128, 512], F32)
            nc.gpsimd.iota(iota_p[:], pattern=[[0, 512]], base=0,
                           channel_multiplier=1,
                           allow_small_or_imprecise_dtypes=True)
            ones_bf = cpool.tile([1, BLK], BF16)
            nc.vector.memset(ones_bf[:], 1.0)

            # WAUG = [W.T | wl | wr]  (wl = W.T a_l, wr = W.T a_r)
            waug = cpool.tile([IN_FEATS, OUT_FEATS + 2], F32)
            with tc.tile_pool(name="wprep", bufs=1) as wpool, \
                 tc.tile_pool(name="wpsum", bufs=2, space="PSUM") as wps:
                w_sb = wpool.tile([OUT_FEATS, IN_FEATS], F32)
                nc.sync.dma_start(out=w_sb[:], in_=Wt[:])
                al_sb = wpool.tile([OUT_FEATS, 1], F32)
                nc.sync.dma_start(out=al_sb[:], in_=av[0:OUT_FEATS, :])
                ar_sb = wpool.tile([OUT_FEATS, 1], F32)
                nc.sync.dma_start(out=ar_sb[:],
                                  in_=av[OUT_FEATS:2 * OUT_FEATS, :])
                wt_ps = wps.tile([IN_FEATS, OUT_FEATS], F32)
                nc.tensor.transpose(out=wt_ps[:], in_=w_sb[:],
                                    identity=ident[0:OUT_FEATS, 0:OUT_FEATS])
                nc.vector.tensor_copy(out=waug[:, 0:OUT_FEATS], in_=wt_ps[:])
                wl_ps = wps.tile([IN_FEATS, 1], F32)
                nc.tensor.matmul(out=wl_ps[:], lhsT=w_sb[:],
                                 rhs=al_sb[:], start=True, stop=True)
                nc.vector.tensor_copy(out=waug[:, 32:33], in_=wl_ps[:])
                wr_ps = wps.tile([IN_FEATS, 1], F32)
                nc.tensor.matmul(out=wr_ps[:], lhsT=w_sb[:],
                                 rhs=ar_sb[:], start=True, stop=True)
                nc.vector.tensor_copy(out=waug[:, 33:34], in_=wr_ps[:])

            # ---------------- phase 1: build zaug / er_sb ----------------
            er_sb = cpool.tile([BLK, ncols], F32)
            zaug_t = zaug.ap().rearrange("(p c) z -> p c z", p=BLK)
            with tc.tile_pool(name="zh", bufs=3) as hpool, \
                 tc.tile_pool(name="zps", bufs=4, space="PSUM") as zps, \
                 tc.tile_pool(name="zrow", bufs=3) as rpool:
                for g in range(nzg):
                    n0 = g * ZG
                    htile = hpool.tile([IN_FEATS, ZG], F32)
                    nc.sync.dma_start(out=htile[:], in_=hT[:, n0:n0 + ZG])
                    zrows = rpool.tile([128, sub, EL], BF16)
                    nc.vector.memset(zrows[:], 0.0)
                    nc.vector.memset(zrows[:, :, C_ONE:C_ONE + 1], 1.0)
                    for s in range(sub):
                        z_ps = zps.tile([128, OUT_FEATS + 2], F32)
                        nc.tensor.matmul(
                            out=z_ps[:],
                            lhsT=htile[:, s * BLK:(s + 1) * BLK],
                            rhs=waug[:], start=True, stop=True)
                        nc.scalar.copy(out=zrows[:, s, 0:OUT_FEATS],
                                       in_=z_ps[:, 0:OUT_FEATS])
                        nc.scalar.copy(
                            out=zrows[:, s, C_ELF:C_ELF + 2].bitcast(F32),
                            in_=z_ps[:, 32:33])
                        nc.vector.tensor_copy(
                            out=er_sb[:, g * sub + s:g * sub + s + 1],
                            in_=z_ps[:, 33:34])
                    nc.sync.dma_start(
                        out=zaug_t[:, g * sub:(g + 1) * sub, :],
                        in_=zrows[:])
                # this core's er, block-column layout [128, nb]
                pid = nc.gpsimd.partition_id()
                er_core = cpool.tile([BLK, nb], F32)
                nc.gpsimd.dma_start(out=er_core[:],
                                    in_=er_sb[:, bass.ts(pid, nb)])

            # ---------------- phase 2: edges ----------------
            with tc.tile_pool(name="ix", bufs=1) as ixpool:
                srcw_sb = ixpool.tile([BLK, NW], I16)
                nc.sync.dma_start(out=srcw_sb[:], in_=srcw[:])
                dcol_sb = ixpool.tile([BLK, T], F32)
                nc.sync.dma_start(out=dcol_sb[:], in_=dstcol[:])
                gcnt_sb = ixpool.tile([1, NQ * nb], mybir.dt.int32)
                nc.sync.dma_start(out=gcnt_sb[:], in_=gcnt[:])

                with tc.tile_pool(name="zg", bufs=4) as zgpool, \
                     tc.tile_pool(name="drw", bufs=3) as drwpool, \
                     tc.tile_pool(name="oht", bufs=3) as ohtpool, \
                     tc.tile_pool(name="ex", bufs=3) as expool, \
                     tc.tile_pool(name="oh", bufs=12) as ohpool, \
                     tc.tile_pool(name="fin", bufs=6) as fpool, \
                     tc.tile_pool(name="ost", bufs=3) as opool, \
                     tc.tile_pool(name="yps", bufs=2, space="PSUM") as ypool, \
                     tc.tile_pool(name="dlb", bufs=2, space="PSUM") as dlbpool, \
                     tc.tile_pool(name="erp", bufs=2, space="PSUM") as erppool, \
                     tc.tile_pool(name="ytp", bufs=2, space="PSUM") as ytpool:
                    NCH = -(-C // 4)  # dlb chunks of <=4 tiles
                    # SWDGE trailing-skip leaves tail slots of zg untouched:
                    # zero the pool buffers once so skipped slots always hold
                    # finite values (0*NaN would poison the PE accumulator)
                    for _ in range(4):
                        zg0 = zgpool.tile([BLK, C, EL], BF16)
                        nc.vector.memset(zg0[:], 0.0)
                    for b in range(nb):
                        y_ps = ypool.tile([NY, BLK], F32)
                        for q in range(NQ):
                            colbase = q * nb * C + b * C
                            w0 = colbase * BLK // 16
                            zg = zgpool.tile([BLK, C, EL], BF16)
                            # one call per run (SWDGE ring cap 1024 idx);
                            # runtime count skips trailing -1 padding
                            assert C * BLK <= 1024
                            nv = nc.gpsimd.value_load(
                                gcnt_sb[0:1, q * nb + b:q * nb + b + 1],
                                min_val=1, max_val=C * BLK)
                            nc.gpsimd.dma_gather(
                                out_ap=zg[:],
                                in_ap=zaug[q * chunk_rows:
                                           (q + 1) * chunk_rows, :],
                                idxs_ap=srcw_sb[:, w0:w0 + C * BLK // 16],
                                num_idxs=C * BLK, num_idxs_reg=nv,
                                elem_size=EL)
                            drow = drwpool.tile([1, C * BLK], BF16)
                            nc.sync.dma_start(
                                out=drow[:],
                                in_=dstrow[colbase:colbase + C, :].rearrange(
                                    "c p -> (c p)")[None, :])
                            erp = erppool.tile([BLK, C], F32)
                            for c in range(NCH):
                                t0, t1 = c * 4, min(c * 4 + 4, C)
                                w = (t1 - t0) * BLK
                                dlb = dlbpool.tile([BLK, 512], F32)
                                nc.tensor.matmul(
                                    out=dlb[:, 0:w],
                                    lhsT=ones_bf[:],
                                    rhs=drow[:, t0 * BLK:t0 * BLK + w],
                                    start=True, stop=True)
                                oht = ohtpool.tile([BLK, 512], F32)
                                nc.vector.tensor_tensor(
                                    out=oht[:, 0:w], in0=dlb[:, 0:w],
                                    in1=iota_p[:, 0:w],
                                    op=mybir.AluOpType.is_equal)
                                for t in range(t0, t1):
                                    nc.tensor.matmul(
                                        out=erp[:, t:t + 1],
                                        lhsT=oht[:, (t - t0) * BLK:
                                                 (t - t0 + 1) * BLK],
                                        rhs=er_core[:, b:b + 1],
                                        start=True, stop=True)
                            ex = expool.tile([BLK, C], F32)
                            sv = expool.tile([BLK, C], F32, tag="sv")
                            nc.vector.tensor_tensor(
                                out=sv[:, :, None],
                                in0=zg[:, :, C_ELF:C_ELF + 2].bitcast(F32),
                                in1=erp[:, :, None],
                                op=mybir.AluOpType.add)
                            # leaky_relu(x) = max(x, 0.2x); the Lrelu ACT
                            # table has a baked-in 0.01 slope
                            nc.vector.tensor_scalar(
                                ex[:], sv[:], NEG_SLOPE, None,
                                mybir.AluOpType.mult)
                            nc.vector.tensor_tensor(
                                out=ex[:], in0=ex[:], in1=sv[:],
                                op=mybir.AluOpType.max)
                            nc.scalar.activation(
                                out=ex[:], in_=ex[:],
                                func=mybir.ActivationFunctionType.Exp)
                            for t in range(C):
                                col = colbase + t
                                oh = ohpool.tile([BLK, BLK], BF16)
                                nc.vector.tensor_scalar(
                                    oh[:], iota_bf[:],
                                    dcol_sb[:, col:col + 1],
                                    ex[:, t:t + 1],
                                    mybir.AluOpType.is_equal,
                                    mybir.AluOpType.mult)
                                nc.tensor.matmul(
                                    out=y_ps[:],
                                    lhsT=zg[:, t, 0:NY],
                                    rhs=oh[:],
                                    start=(q == 0 and t == 0),
                                    stop=(q == NQ - 1 and t == C - 1))
                        yc = fpool.tile([NY, BLK], F32)
                        nc.scalar.copy(out=yc[:], in_=y_ps[:])
                        yt = ytpool.tile([BLK, NY], F32)
                        nc.tensor.transpose(out=yt[:], in_=yc[:],
                                            identity=ident[0:NY, 0:NY])
                        den = fpool.tile([BLK, 1], F32)
                        nc.vector.tensor_scalar(
                            den[:], yt[:, 32:33], 1e-16, None,
                            mybir.AluOpType.max)
                        rden = fpool.tile([BLK, 1], F32)
                        nc.vector.reciprocal(out=rden[:], in_=den[:])
                        ost = opool.tile([BLK, OUT_FEATS], F32)
                        nc.vector.tensor_scalar(
                            ost[:], yt[:, 0:OUT_FEATS], rden[:], None,
                            mybir.AluOpType.mult)
                        nc.sync.dma_start(
                            out=out[b * BLK:(b + 1) * BLK, :], in_=ost[:])

    nc.compile()
    return nc


def _prep(h, W, a, src, dst, nb=NB, n_nodes=N_NODES):
    """Host-side sharding / index layout (integer index manipulation and
    zero-padding only - all floating-point math runs on device)."""
    import ml_dtypes

    core_nodes = nb * BLK
    npad = N_CORES * core_nodes
    ncols = npad // BLK
    chunk_rows = (BLK // NQ) * ncols

    h = np.asarray(h, dtype=np.float32)
    W = np.ascontiguousarray(np.asarray(W, dtype=np.float32))
    a = np.asarray(a, dtype=np.float32).reshape(-1)
    src = np.asarray(src, dtype=np.int64)
    dst = np.asarray(dst, dtype=np.int64)

    hT = np.zeros((IN_FEATS, npad), dtype=np.float32)
    hT[:, :n_nodes] = h.T
    av = np.ascontiguousarray(a.reshape(-1, 1), dtype=np.float32)

    core = dst // core_nodes
    b_of = (dst % core_nodes) // BLK
    q_of = (src % BLK) // (BLK // NQ)
    src_t = (src % BLK) * ncols + src // BLK  # tiled z-table row
    grp = (core * NQ + q_of) * nb + b_of
    # within each run, order edges by table row for DRAM locality
    order = np.lexsort((src_t, grp))
    gs = grp[order]
    ss_t = src_t[order]
    ds = dst[order]

    counts = np.bincount(gs, minlength=N_CORES * NQ * nb)
    C = int(max(1, -(-counts.max() // BLK)))
    T = NQ * nb * C
    NW = T * BLK // 16

    starts = np.zeros(N_CORES * NQ * nb + 1, dtype=np.int64)
    np.cumsum(counts, out=starts[1:])
    rank = np.arange(len(ds)) - starts[gs]
    gloc = gs % (NQ * nb)  # within-core group index (q * nb + b)
    slot = gloc * (C * BLK) + rank  # slot within the core's edge buffer

    src_i16 = (ss_t - q_of[order] * chunk_rows).astype(np.int16)
    dl_all = (ds % core_nodes - b_of[order] * BLK).astype(np.float32)

    srcw = np.zeros((N_CORES, BLK, NW), dtype=np.int16)
    gcnt = np.zeros((N_CORES, 1, NQ * nb), dtype=np.int32)
    dstcol = np.zeros((N_CORES, BLK, T), dtype=np.float32)
    dstrow = np.zeros((N_CORES, T, BLK), dtype=ml_dtypes.bfloat16)
    for k in range(N_CORES):
        m = core[order] == k
        sl = slot[m]
        sflat = np.full(T * BLK, -1, dtype=np.int16)
        dflat = np.full(T * BLK, -1.0, dtype=np.float32)
        sflat[sl] = src_i16[m]
        dflat[sl] = dl_all[m]
        ck = counts[k * NQ * nb:(k + 1) * NQ * nb].astype(np.int64)
        empty = np.nonzero(ck == 0)[0]
        sflat[empty * (C * BLK)] = 0  # keep >=1 valid index per call
        gcnt[k, 0] = np.maximum(ck, 1).astype(np.int32)
        # wrapped-16, replicated over the 8 gpsimd groups
        srcw[k] = np.tile(sflat.reshape(-1, 16).T, (8, 1))
        dstcol[k] = dflat.reshape(T, BLK).T
        dstrow[k] = dflat.reshape(T, BLK).astype(ml_dtypes.bfloat16)
    return hT, W, av, srcw, dstcol, dstrow, gcnt, C


def _in_maps(prep):
    hT, Wm, av, srcw, dstcol, dstrow, gcnt = prep[:-1]
    return [
        {"hT": hT, "Wt": Wm, "av": av, "srcw": srcw[k], "dstcol": dstcol[k],
         "dstrow": dstrow[k], "gcnt": gcnt[k]}
        for k in range(N_CORES)
    ]


def kernel(h, W, a, src, dst):
    prep = _prep(h, W, a, src, dst)
    C = prep[-1]
    if (C, 1) not in _cache:
        _cache[C, 1] = _build(C)
    nc = _cache[C, 1]
    in_maps = _in_maps(prep)
    res = run_bass_kernel_spmd(nc, in_maps, list(range(N_CORES)))
    outs = [res.results[k]["out"] for k in range(N_CORES)]
    full = np.concatenate(outs, axis=0)[:N_NODES]
    return np.ascontiguousarray(full, dtype=np.float32)
